# revision 1
# baseline (speedup 1.0000x reference)
"""Trainium2 Bass kernel for the ColorMemory block.

Sharding: data-parallel over batch b across 8 NeuronCores (one batch element
per core); all weights and the 512-row memory bank replicated per core.

Per-core math (all f32 storage; matmuls/transposes run as float32r = FP22):
  xt   = x.T                              [hw, c]     (per 128-token subtile)
  z1   = standardize(xt)                  (LN1 affine folded into q_w)
  qT   = qW'^T @ z1^T                     [e, s]  feature-major
  l    = qT^T @ semT                      [s, n]  logits, softmax over n
  p    = exp(l - max), denom accumulated by ACT
  cp   = (p^T)^T @ color_embed / denom    [s, ce]
  y    = [xt | cp],  z2 = standardize(y)  (LN2 affine folded into fc1)
  h    = gelu(z2 @ fc1'),  mlp = h @ fc2
  v    = y2 + mlp,  z3 = standardize(v)   (LN3 affine folded into conv)
  outT = conv'^T @ z3^T                   [c, s]  -> native output layout

Three passes over the 32 token-subtiles keep ACT on a single table set per
pass (exp/ln -> gelu -> ln/exp), separated by scheduler-only fences.
"""

import numpy as np
from contextlib import ExitStack

import concourse.bass as bass
import concourse.tile as tile
from concourse import bacc, mybir
from concourse.bass_utils import run_bass_kernel_spmd
from concourse.masks import make_identity

F32 = mybir.dt.float32
F32R = mybir.dt.float32r
I32 = mybir.dt.int32
AF = mybir.ActivationFunctionType
OP = mybir.AluOpType

N_CORES = 8
B, C, H, W = 8, 256, 64, 64
S = H * W              # 4096 tokens per core
NCOL = 512             # memory bank rows
SEMD = 512             # semantic dim
E = 768                # embed dim
CE = 256               # color embed dim
D2 = C + CE            # 512
EPS = 1e-5
P = 128

N_SUB = S // P         # 32 subtiles of 128 tokens
N_PAIR = N_SUB // 2    # 16 pairs (pairing keeps matmul free dims >= 256)

EC = E // P            # 6 e-chunks
CC = C // P            # 2 c-chunks
DC = D2 // P           # 4 f-chunks of the concat dim
NC_ = NCOL // P        # 4 n-chunks
VC = 256 // P          # 2 vocab chunks of the a/b embed tables


RSQRT_MAGIC = 0x5F3759DF


def _rstd_ln_exp(nc, pool, var_ap, eps_col):
    """rstd = (var+eps)^-0.5 via bit-magic + 2 Newton steps, all on DVE
    (avoids ACT table-set thrashing between Ln/Exp/Gelu)."""
    a = pool.tile([P, 1], F32)
    nc.vector.tensor_scalar(out=a[:], in0=var_ap, scalar1=float(EPS),
                            scalar2=None, op0=OP.add)
    tb = pool.tile([P, 1], I32)
    nc.vector.tensor_scalar(out=tb[:], in0=a[:].bitcast(I32), scalar1=1,
                            scalar2=None, op0=OP.logical_shift_right)
    nb = pool.tile([P, 1], I32)
    nc.vector.tensor_scalar(out=nb[:], in0=tb[:], scalar1=RSQRT_MAGIC,
                            scalar2=-1, op0=OP.subtract, op1=OP.mult)
    y = nb[:].bitcast(F32)
    for _ in range(2):
        t = pool.tile([P, 1], F32, tag="nt_t")
        nc.vector.tensor_tensor(out=t[:], in0=y, in1=y, op=OP.mult)
        nc.vector.tensor_tensor(out=t[:], in0=t[:], in1=a[:], op=OP.mult)
        nc.vector.tensor_scalar(out=t[:], in0=t[:], scalar1=-0.5,
                                scalar2=1.5, op0=OP.mult, op1=OP.add)
        y2 = pool.tile([P, 1], F32, tag="nt_y")
        nc.vector.tensor_tensor(out=y2[:], in0=y, in1=t[:], op=OP.mult)
        y = y2[:]
    return y2


def _standardize(nc, pool, out_ap, in_ap, stats_pool, eps_col):
    """out = (in - mean(in)) * rsqrt(var(in)+eps) along the free axis."""
    stats = stats_pool.tile([P, nc.vector.BN_STATS_DIM], F32)
    nc.vector.bn_stats(out=stats[:], in_=in_ap)
    mv = stats_pool.tile([P, nc.vector.BN_AGGR_DIM], F32)
    nc.vector.bn_aggr(out=mv[:], in_=stats[:])
    rstd = _rstd_ln_exp(nc, stats_pool, mv[:, 1:2], eps_col)
    nc.vector.tensor_scalar(
        out=out_ap, in0=in_ap,
        scalar1=mv[:, 0:1], scalar2=rstd[:],
        op0=OP.subtract, op1=OP.mult,
    )


def _standardize_sqrt(nc, pool, out_ap, in_ap, stats_pool, eps_col):
    """Pass-3 standardize: rstd via ACT Sqrt + DVE reciprocal."""
    stats = stats_pool.tile([P, nc.vector.BN_STATS_DIM], F32)
    nc.vector.bn_stats(out=stats[:], in_=in_ap)
    mv = stats_pool.tile([P, nc.vector.BN_AGGR_DIM], F32)
    nc.vector.bn_aggr(out=mv[:], in_=stats[:])
    sd = stats_pool.tile([P, 1], F32)
    nc.scalar.activation(out=sd[:], in_=mv[:, 1:2], func=AF.Sqrt, bias=eps_col)
    rstd = stats_pool.tile([P, 1], F32)
    nc.vector.reciprocal(out=rstd[:], in_=sd[:])
    nc.vector.tensor_scalar(
        out=out_ap, in0=in_ap,
        scalar1=mv[:, 0:1], scalar2=rstd[:],
        op0=OP.subtract, op1=OP.mult,
    )


def _bn_to(nc, stats_pool, in_ap, mv_slot_ap):
    st = stats_pool.tile([P, nc.vector.BN_STATS_DIM], F32, tag="bnst")
    nc.vector.bn_stats(out=st[:], in_=in_ap)
    nc.vector.bn_aggr(out=mv_slot_ap, in_=st[:])


def _newton2(nc, pool, var2_ap):
    """rstd [P,2] for a pair of variance columns, one Newton chain."""
    a = pool.tile([P, 2], F32, tag="nw_a")
    nc.vector.tensor_scalar(out=a[:], in0=var2_ap, scalar1=float(EPS),
                            scalar2=None, op0=OP.add)
    tb = pool.tile([P, 2], I32, tag="nw_b")
    nc.vector.tensor_scalar(out=tb[:], in0=a[:].bitcast(I32), scalar1=1,
                            scalar2=None, op0=OP.logical_shift_right)
    nb = pool.tile([P, 2], I32, tag="nw_c")
    nc.vector.tensor_scalar(out=nb[:], in0=tb[:], scalar1=RSQRT_MAGIC,
                            scalar2=-1, op0=OP.subtract, op1=OP.mult)
    y = nb[:].bitcast(F32)
    for _ in range(2):
        t = pool.tile([P, 2], F32, tag="nw_t")
        nc.vector.tensor_tensor(out=t[:], in0=y, in1=y, op=OP.mult)
        nc.vector.tensor_tensor(out=t[:], in0=t[:], in1=a[:], op=OP.mult)
        nc.vector.tensor_scalar(out=t[:], in0=t[:], scalar1=-0.5,
                                scalar2=1.5, op0=OP.mult, op1=OP.add)
        y2 = pool.tile([P, 2], F32, tag="nw_y")
        nc.vector.tensor_tensor(out=y2[:], in0=y, in1=t[:], op=OP.mult)
        y = y2[:]
    return y2


def _drain(nc, out_ap, psum_ap, bias_col=None):
    """PSUM -> SBUF copy, optionally adding a per-partition bias column."""
    if bias_col is None:
        nc.any.tensor_copy(out=out_ap, in_=psum_ap)
    else:
        nc.any.tensor_scalar(
            out=out_ap, in0=psum_ap, scalar1=bias_col, scalar2=None, op0=OP.add
        )


def build_bass(flags):
    """Build the SPMD program. flags: which optional bias paths are live."""
    nc = bacc.Bacc(
        "TRN2",
        target_bir_lowering=False,
        debug=False,
        enable_asserts=False,
        num_devices=N_CORES,
    )

    # ---- DRAM I/O (per-core shapes) ----
    x_d = nc.dram_tensor("x", [C, S], F32R, kind="ExternalInput").ap()
    cls_d = nc.dram_tensor("cls", [4], F32, kind="ExternalInput").ap()
    cc_d = nc.dram_tensor("cc_idx", [4, NCOL, 2], I32, kind="ExternalInput").ap()
    cen_d = nc.dram_tensor("centers", [NCOL, SEMD], F32R, kind="ExternalInput").ap()
    aemb_d = nc.dram_tensor("aemb", [256, CE], F32R, kind="ExternalInput").ap()
    bemb_d = nc.dram_tensor("bemb", [256, CE], F32R, kind="ExternalInput").ap()
    cew_d = nc.dram_tensor("cew", [4, 2 * CE, CE], F32R, kind="ExternalInput").ap()
    semw_d = nc.dram_tensor("semw", [SEMD, E], F32R, kind="ExternalInput").ap()
    qw_d = nc.dram_tensor("qw", [C, E], F32R, kind="ExternalInput").ap()
    fc1_d = nc.dram_tensor("fc1", [D2, D2], F32R, kind="ExternalInput").ap()
    fc2_d = nc.dram_tensor("fc2", [D2, D2], F32R, kind="ExternalInput").ap()
    conv_d = nc.dram_tensor("conv", [D2, C], F32R, kind="ExternalInput").ap()
    opt = {}
    if flags["qb"]:
        opt["qb"] = nc.dram_tensor("qb", [E, 1], F32, kind="ExternalInput").ap()
    if flags["semb"]:
        opt["semb"] = nc.dram_tensor("semb", [E, 1], F32, kind="ExternalInput").ap()
    if flags["ceb"]:
        opt["ceb"] = nc.dram_tensor("ceb", [4, CE], F32, kind="ExternalInput").ap()
    if flags["c1"]:
        opt["c1"] = nc.dram_tensor("c1b", [P, D2], F32, kind="ExternalInput").ap()
    if flags["fc2b"]:
        opt["fc2b"] = nc.dram_tensor("fc2b", [P, D2], F32, kind="ExternalInput").ap()
    if flags["ln2w"]:
        opt["ln2w"] = nc.dram_tensor("ln2w", [P, D2], F32, kind="ExternalInput").ap()
    if flags["ln2b"]:
        opt["ln2b"] = nc.dram_tensor("ln2b", [P, D2], F32, kind="ExternalInput").ap()
    if flags["ccb"]:
        opt["ccb"] = nc.dram_tensor("ccb", [C, 1], F32, kind="ExternalInput").ap()
    out_d = nc.dram_tensor("out", [C, S], F32, kind="ExternalOutput").ap()

    with tile.TileContext(nc) as tc, ExitStack() as ctx:
        # ---- persistent SBUF pools ----
        wpool = ctx.enter_context(tc.tile_pool(name="weights", bufs=1))
        z2pool = ctx.enter_context(tc.tile_pool(name="z2store", bufs=N_SUB))

        ident_f32 = wpool.tile([P, P], F32)
        make_identity(nc, ident_f32[:])
        ident = wpool.tile([P, P], F32R)
        nc.vector.tensor_copy(out=ident[:], in_=ident_f32[:])
        eps_col = wpool.tile([P, 1], F32)
        nc.vector.memset(eps_col[:], EPS)

        semw_sb = wpool.tile([P, SEMD // P, E], F32R)
        nc.sync.dma_start(
            out=semw_sb[:], in_=semw_d.rearrange("(k p) e -> p k e", p=P)
        )
        qw_sb = wpool.tile([P, CC, E], F32R)
        nc.sync.dma_start(out=qw_sb[:], in_=qw_d.rearrange("(k p) e -> p k e", p=P))
        fc1_sb = wpool.tile([P, DC, D2], F32R)
        nc.sync.dma_start(out=fc1_sb[:], in_=fc1_d.rearrange("(k p) e -> p k e", p=P))
        fc2_sb = wpool.tile([P, DC, D2], F32R)
        nc.sync.dma_start(out=fc2_sb[:], in_=fc2_d.rearrange("(k p) e -> p k e", p=P))
        conv_sb = wpool.tile([P, DC, C], F32R)
        nc.sync.dma_start(out=conv_sb[:], in_=conv_d.rearrange("(k p) e -> p k e", p=P))

        bias_sb = {}
        for key, rows in (("qb", E), ("semb", E), ("ccb", C)):
            if flags[key if key != "ccb" else "ccb"] and key in opt:
                t = wpool.tile([P, rows // P, 1], F32)
                nc.sync.dma_start(
                    out=t[:], in_=opt[key].rearrange("(k p) o -> p k o", p=P)
                )
                bias_sb[key] = t
        for key in ("c1", "fc2b", "ln2w", "ln2b"):
            if flags[key]:
                t = wpool.tile([P, D2], F32)
                nc.sync.dma_start(out=t[:], in_=opt[key])
                bias_sb[key] = t

        semT_sb = wpool.tile([P, EC, NCOL], F32R)
        colemb_sb = wpool.tile([P, NC_, CE], F32R)

        # ================= precompute =================
        with (
            tc.tile_pool(name="prep", bufs=2) as prep,
            tc.tile_pool(name="prep1", bufs=1) as prep1,
            tc.tile_pool(name="ptp", bufs=3, space="PSUM") as ptp,
            tc.tile_pool(name="pacc", bufs=2, space="PSUM") as pacc,
        ):
            # centers^T, then semT = semw^T @ centers^T  (feature-major sem)
            cenT_sb = prep1.tile([P, SEMD // P, NCOL], F32R)
            for ncc in range(NC_):
                cen_sb = prep.tile([P, SEMD], F32R, tag="cen")
                nc.sync.dma_start(
                    out=cen_sb[:], in_=cen_d[ncc * P:(ncc + 1) * P, :]
                )
                for dcc in range(SEMD // P):
                    tp = ptp.tile([P, P], F32R, tag="tp")
                    nc.tensor.transpose(
                        out=tp[:],
                        in_=cen_sb[:, dcc * P:(dcc + 1) * P],
                        identity=ident[:],
                    )
                    _drain(nc, cenT_sb[:, dcc, ncc * P:(ncc + 1) * P], tp[:])
            for ec in range(EC):
                ps = pacc.tile([P, NCOL], F32, tag="sem")
                for dcc in range(SEMD // P):
                    nc.tensor.matmul(
                        out=ps[:],
                        lhsT=semw_sb[:, dcc, ec * P:(ec + 1) * P],
                        rhs=cenT_sb[:, dcc, :],
                        start=(dcc == 0), stop=(dcc == SEMD // P - 1),
                    )
                _drain(nc, semT_sb[:, ec, :], ps[:],
                       bias_sb["semb"][:, ec, :] if flags["semb"] else None)

            # gather color-center embeddings, transpose to feature-major
            abT = []
            for i in range(4):
                abT_i = prep1.tile([P, 2 * CE // P, NCOL], F32R, tag=f"abT{i}")
                for ncc in range(NC_):
                    ab_sb = prep.tile([P, 2 * CE], F32R, tag="ab")
                    for ch, emb_d in ((0, aemb_d), (1, bemb_d)):
                        idx = prep.tile([P, 1], I32, tag="idx")
                        nc.sync.dma_start(
                            out=idx[:],
                            in_=cc_d[i, ncc * P:(ncc + 1) * P, ch:ch + 1],
                        )
                        nc.gpsimd.indirect_dma_start(
                            out=ab_sb[:, ch * CE:(ch + 1) * CE],
                            out_offset=None,
                            in_=emb_d,
                            in_offset=bass.IndirectOffsetOnAxis(
                                ap=idx[:, :1], axis=0
                            ),
                        )
                    for fcc in range(2 * CE // P):
                        tp = ptp.tile([P, P], F32R, tag="tp")
                        nc.tensor.transpose(
                            out=tp[:],
                            in_=ab_sb[:, fcc * P:(fcc + 1) * P],
                            identity=ident[:],
                        )
                        _drain(nc, abT_i[:, fcc, ncc * P:(ncc + 1) * P], tp[:])
                abT.append(abT_i)

            # cls-scaled ce_w, then color_embed = sum_i ab_i @ (cls_i ce_w_i)
            cewS = prep1.tile([P, 4, 2 * CE // P, CE], F32R)
            cls_col = []
            for i in range(4):
                cb = prep.tile([P, 1], F32, tag="clsb")
                nc.sync.dma_start(
                    out=cb[:],
                    in_=bass.AP(tensor=cls_d.tensor, offset=i, ap=[[0, P], [1, 1]]),
                )
                cls_col.append(cb)
                for fcc in range(2 * CE // P):
                    nc.sync.dma_start(
                        out=cewS[:, i, fcc, :],
                        in_=cew_d[i, fcc * P:(fcc + 1) * P, :],
                    )
                    nc.vector.tensor_scalar(
                        out=cewS[:, i, fcc, :], in0=cewS[:, i, fcc, :],
                        scalar1=cls_col[i][:], scalar2=None, op0=OP.mult,
                    )
            ceb_sb = None
            ones_row = None
            if flags["ceb"]:
                ceb_raw = prep1.tile([4, CE], F32R, tag="cebr")
                nc.sync.dma_start(out=ceb_raw[:], in_=opt["ceb"])
                cls4 = prep1.tile([4, 1], F32R, tag="cls4")
                nc.sync.dma_start(
                    out=cls4[:],
                    in_=bass.AP(tensor=cls_d.tensor, offset=0, ap=[[1, 4], [1, 1]]),
                )
                ceb_ps = pacc.tile([1, CE], F32, tag="cebp")
                nc.tensor.matmul(out=ceb_ps[:], lhsT=cls4[:],
                                 rhs=ceb_raw[:], start=True, stop=True)
                ceb_sb = prep1.tile([1, CE], F32R, tag="ceb1")
                _drain(nc, ceb_sb[:], ceb_ps[:])
                ones_row = prep1.tile([1, P], F32R, tag="ones")
                nc.vector.memset(ones_row[:], 1.0)
            mm_list = [(i, fcc) for i in range(4) for fcc in range(2 * CE // P)]
            for ncc in range(NC_):
                ps = pacc.tile([P, CE], F32, tag="ce")
                for j, (i, fcc) in enumerate(mm_list):
                    nc.tensor.matmul(
                        out=ps[:],
                        lhsT=abT[i][:, fcc, ncc * P:(ncc + 1) * P],
                        rhs=cewS[:, i, fcc, :],
                        start=(j == 0),
                        stop=(j == len(mm_list) - 1) and not flags["ceb"],
                    )
                if flags["ceb"]:
                    nc.tensor.matmul(out=ps[:], lhsT=ones_row[:],
                                     rhs=ceb_sb[:], start=False, stop=True)
                _drain(nc, colemb_sb[:, ncc, :], ps[:])

        tc.no_sync_barrier()

        # ================= pass 1: attention =================
        with (
            tc.tile_pool(name="p1pair", bufs=3) as pairp,
            tc.tile_pool(name="p1work", bufs=5) as work,
            tc.tile_pool(name="p1y", bufs=6) as ypool,
            tc.tile_pool(name="p1stats", bufs=40) as stats,
            tc.tile_pool(name="p1tp", bufs=2, space="PSUM") as p_tp,
            tc.tile_pool(name="p1q", bufs=2, space="PSUM") as p_q,
            tc.tile_pool(name="p1l", bufs=2, space="PSUM") as p_l,
            tc.tile_pool(name="p1cp", bufs=2, space="PSUM") as p_cp,
        ):
            for pp in range(N_PAIR):
                xzT_pair = pairp.tile([P, CC, 2 * P], F32R, tag="xzT")
                y_pair = []
                mv1 = stats.tile([P, 2, nc.vector.BN_AGGR_DIM], F32,
                                 tag="mv1")
                for half in range(2):
                    t = 2 * pp + half
                    y_t = ypool.tile([P, D2], F32, tag="y")
                    y_pair.append(y_t)
                    for ccc in range(CC):
                        xn = work.tile([P, P], F32R, tag="xn")
                        nc.sync.dma_start(
                            out=xn[:],
                            in_=x_d[ccc * P:(ccc + 1) * P, t * P:(t + 1) * P],
                        )
                        tp = p_tp.tile([P, P], F32R, tag="tp")
                        nc.tensor.transpose(
                            out=tp[:], in_=xn[:], identity=ident[:]
                        )
                        _drain(nc, y_t[:, ccc * P:(ccc + 1) * P], tp[:])
                    _bn_to(nc, stats, y_t[:, 0:C], mv1[:, half, :])
                rstd1 = _newton2(nc, stats, mv1[:, :, 1])
                for half in range(2):
                    y_t = y_pair[half]
                    xz = work.tile([P, C], F32R, tag="xz")
                    nc.vector.tensor_scalar(
                        out=xz[:], in0=y_t[:, 0:C],
                        scalar1=mv1[:, half, 0:1],
                        scalar2=rstd1[:, half:half + 1],
                        op0=OP.subtract, op1=OP.mult,
                    )
                    for ccc in range(CC):
                        tp = p_tp.tile([P, P], F32R, tag="tp")
                        nc.tensor.transpose(
                            out=tp[:],
                            in_=xz[:, ccc * P:(ccc + 1) * P],
                            identity=ident[:],
                        )
                        _drain(
                            nc,
                            xzT_pair[:, ccc, half * P:(half + 1) * P],
                            tp[:],
                        )
                # qT for the pair (free dim 256 keeps f32r at full rate)
                qT_pair = pairp.tile([P, EC, 2 * P], F32R, tag="qT")
                for eg in range(EC // 2):
                    ps = p_q.tile([P, 2, 2 * P], F32, tag="q")
                    for sub in range(2):
                        ec = 2 * eg + sub
                        for ccc in range(CC):
                            nc.tensor.matmul(
                                out=ps[:, sub, :],
                                lhsT=qw_sb[:, ccc, ec * P:(ec + 1) * P],
                                rhs=xzT_pair[:, ccc, :],
                                start=(ccc == 0), stop=(ccc == CC - 1),
                            )
                    if flags["qb"]:
                        for sub in range(2):
                            _drain(nc, qT_pair[:, 2 * eg + sub, :],
                                   ps[:, sub, :],
                                   bias_sb["qb"][:, 2 * eg + sub, :])
                    else:
                        _drain(nc, qT_pair[:, 2 * eg:2 * eg + 2, :], ps[:])
                for half in range(2):
                    t = 2 * pp + half
                    y_t = y_pair[half]
                    ps_l = p_l.tile([P, NCOL], F32, tag="l")
                    for ec in range(EC):
                        nc.tensor.matmul(
                            out=ps_l[:],
                            lhsT=qT_pair[:, ec, half * P:(half + 1) * P],
                            rhs=semT_sb[:, ec, :],
                            start=(ec == 0), stop=(ec == EC - 1),
                        )
                    negmax = stats.tile([P, 1], F32, tag="negmax")
                    nc.vector.reduce_max(
                        out=negmax[:], in_=ps_l[:],
                        axis=mybir.AxisListType.X, negate=True,
                    )
                    p_sb = work.tile([P, NCOL], F32R, tag="p")
                    denom = stats.tile([P, 1], F32, tag="denom")
                    nc.scalar.activation(
                        out=p_sb[:], in_=ps_l[:], func=AF.Exp,
                        bias=negmax[:], accum_out=denom[:],
                    )
                    recip = stats.tile([P, 1], F32, tag="recip")
                    nc.vector.reciprocal(out=recip[:], in_=denom[:])
                    pT = work.tile([P, NC_, P], F32R, tag="pT")
                    tp4 = p_tp.tile([P, NC_, P], F32R, tag="tp")
                    for ncc in range(NC_):
                        nc.tensor.transpose(
                            out=tp4[:, ncc, :],
                            in_=p_sb[:, ncc * P:(ncc + 1) * P],
                            identity=ident[:],
                        )
                    _drain(nc, pT[:], tp4[:])
                    ps_cp = p_cp.tile([P, CE], F32, tag="cp")
                    for ncc in range(NC_):
                        nc.tensor.matmul(
                            out=ps_cp[:],
                            lhsT=pT[:, ncc, :],
                            rhs=colemb_sb[:, ncc, :],
                            start=(ncc == 0), stop=(ncc == NC_ - 1),
                        )
                    # normalized color prior straight into the concat tile
                    nc.vector.tensor_scalar(
                        out=y_t[:, C:D2], in0=ps_cp[:],
                        scalar1=recip[:], scalar2=None, op0=OP.mult,
                    )
                    # LN2 stats; rstd + apply batched after both halves
                    if half == 0:
                        mv2 = stats.tile([P, 2, nc.vector.BN_AGGR_DIM], F32,
                                         tag="mv2")
                    _bn_to(nc, stats, y_t[:], mv2[:, half, :])
                rstd2 = _newton2(nc, stats, mv2[:, :, 1])
                for half in range(2):
                    y_t = y_pair[half]
                    z2_t = z2pool.tile([P, D2], F32R, tag="z2")
                    nc.vector.tensor_scalar(
                        out=z2_t[:], in0=y_t[:],
                        scalar1=mv2[:, half, 0:1],
                        scalar2=rstd2[:, half:half + 1],
                        op0=OP.subtract, op1=OP.mult,
                    )
                    if flags["ln2w"]:
                        nc.vector.tensor_tensor(
                            out=z2_t[:], in0=z2_t[:], in1=bias_sb["ln2w"][:],
                            op=OP.mult,
                        )
                    if flags["ln2b"]:
                        nc.vector.tensor_tensor(
                            out=z2_t[:], in0=z2_t[:], in1=bias_sb["ln2b"][:],
                            op=OP.add,
                        )
                    if pp == 0 and half == 0:
                        z2_tiles = []
                    z2_tiles.append(z2_t)

        tc.no_sync_barrier()

        # ================= pass 2: MLP (gelu) =================
        with (
            tc.tile_pool(name="p2work", bufs=6) as work2,
            tc.tile_pool(name="p2tp", bufs=3, space="PSUM") as p2_tp,
            tc.tile_pool(name="p2h", bufs=3, space="PSUM") as p2_h,
            tc.tile_pool(name="p2m", bufs=2, space="PSUM") as p2_m,
        ):
            for t in range(N_SUB):
                z2_t = z2_tiles[t]
                z2T = work2.tile([P, DC, P], F32R, tag="z2T")
                tpz = p2_tp.tile([P, DC, P], F32R, tag="tp")
                for fcc in range(DC):
                    nc.tensor.transpose(
                        out=tpz[:, fcc, :],
                        in_=z2_t[:, fcc * P:(fcc + 1) * P],
                        identity=ident[:],
                    )
                _drain(nc, z2T[:], tpz[:])
                ps_h = p2_h.tile([P, D2], F32, tag="h")
                for fcc in range(DC):
                    nc.tensor.matmul(
                        out=ps_h[:],
                        lhsT=z2T[:, fcc, :],
                        rhs=fc1_sb[:, fcc, :],
                        start=(fcc == 0), stop=(fcc == DC - 1),
                    )
                if flags["c1"]:
                    nc.vector.tensor_tensor(
                        out=ps_h[:], in0=ps_h[:], in1=bias_sb["c1"][:], op=OP.add
                    )
                h_sb = work2.tile([P, D2], F32R, tag="h")
                nc.scalar.activation(out=h_sb[:], in_=ps_h[:], func=AF.Gelu)
                hT = work2.tile([P, DC, P], F32R, tag="hT")
                tph = p2_tp.tile([P, DC, P], F32R, tag="tp")
                for fcc in range(DC):
                    nc.tensor.transpose(
                        out=tph[:, fcc, :],
                        in_=h_sb[:, fcc * P:(fcc + 1) * P],
                        identity=ident[:],
                    )
                _drain(nc, hT[:], tph[:])
                ps_m = p2_m.tile([P, D2], F32, tag="m")
                for fcc in range(DC):
                    nc.tensor.matmul(
                        out=ps_m[:],
                        lhsT=hT[:, fcc, :],
                        rhs=fc2_sb[:, fcc, :],
                        start=(fcc == 0), stop=(fcc == DC - 1),
                    )
                if flags["fc2b"]:
                    nc.vector.tensor_tensor(
                        out=ps_m[:], in0=ps_m[:], in1=bias_sb["fc2b"][:], op=OP.add
                    )
                # v = y_ln2 + mlp, stored back in place of z2
                nc.vector.tensor_tensor(
                    out=z2_t[:], in0=z2_t[:], in1=ps_m[:], op=OP.add
                )

        tc.no_sync_barrier()

        # ================= pass 3: LN3 + output conv =================
        with (
            tc.tile_pool(name="p3work", bufs=6) as work3,
            tc.tile_pool(name="p3stats", bufs=24) as stats3,
            tc.tile_pool(name="p3tp", bufs=4, space="PSUM") as p3_tp,
            tc.tile_pool(name="p3o", bufs=4, space="PSUM") as p3_o,
        ):
            for pp in range(N_PAIR):
                z3T_pair = work3.tile([P, DC, 2 * P], F32R, tag="z3T")
                for half in range(2):
                    t = 2 * pp + half
                    v_t = z2_tiles[t]
                    z3 = work3.tile([P, D2], F32R, tag="z3")
                    _standardize_sqrt(nc, work3, z3[:], v_t[:], stats3, eps_col)
                    tp3 = p3_tp.tile([P, DC, P], F32R, tag="tp")
                    for fcc in range(DC):
                        nc.tensor.transpose(
                            out=tp3[:, fcc, :],
                            in_=z3[:, fcc * P:(fcc + 1) * P],
                            identity=ident[:],
                        )
                    _drain(
                        nc,
                        z3T_pair[:, :, half * P:(half + 1) * P],
                        tp3[:],
                    )
                out_pair = work3.tile([P, CC, 2 * P], F32, tag="op")
                pso = p3_o.tile([P, CC, 2 * P], F32, tag="o")
                for ccc in range(CC):
                    for fcc in range(DC):
                        nc.tensor.matmul(
                            out=pso[:, ccc, :],
                            lhsT=conv_sb[:, fcc, ccc * P:(ccc + 1) * P],
                            rhs=z3T_pair[:, fcc, :],
                            start=(fcc == 0), stop=(fcc == DC - 1),
                        )
                if flags["ccb"]:
                    for ccc in range(CC):
                        _drain(nc, out_pair[:, ccc, :], pso[:, ccc, :],
                               bias_sb["ccb"][:, ccc, :])
                else:
                    _drain(nc, out_pair[:], pso[:])
                for ccc in range(CC):
                    nc.sync.dma_start(
                        out=out_d[ccc * P:(ccc + 1) * P,
                                  pp * 2 * P:(pp + 1) * 2 * P],
                        in_=out_pair[:, ccc, :],
                    )

    nc.compile()
    return nc


_CACHE = {}


def _prep_inputs_impl(x, cls, color_centers, semantic_centers, a_embed, b_embed,
           ce_w, ce_b, sem_w, sem_b, q_w, q_b,
           n1_w, n1_b, n2_w, n2_b, n3_w, n3_b,
           fc1_w, fc1_b, fc2_w, fc2_b, conv_w, conv_b):
    x = np.ascontiguousarray(np.asarray(x, np.float32))
    cls = np.asarray(cls, np.float32)
    color_centers = np.ascontiguousarray(np.asarray(color_centers, np.int32))
    semantic_centers = np.ascontiguousarray(np.asarray(semantic_centers, np.float32))

    f32 = lambda a: np.asarray(a, np.float32)
    a_embed, b_embed = f32(a_embed), f32(b_embed)
    ce_w, ce_b = f32(ce_w), f32(ce_b)
    sem_w, sem_b = f32(sem_w), f32(sem_b)
    q_w, q_b = f32(q_w), f32(q_b)
    n1_w, n1_b = f32(n1_w), f32(n1_b)
    n2_w, n2_b = f32(n2_w), f32(n2_b)
    n3_w, n3_b = f32(n3_w), f32(n3_b)
    fc1_w, fc1_b = f32(fc1_w), f32(fc1_b)
    fc2_w, fc2_b = f32(fc2_w), f32(fc2_b)
    conv_w, conv_b = f32(conv_w), f32(conv_b)

    # host-side weight folding (layernorm affines into adjacent matmuls)
    qw_f = np.ascontiguousarray(n1_w[:, None] * q_w)
    qb_f = q_b + n1_b @ q_w
    fc1_f = np.ascontiguousarray(n2_w[:, None] * fc1_w)
    c1_f = fc1_b + n2_b @ fc1_w
    conv_f = np.ascontiguousarray(n3_w[:, None] * conv_w)
    ccb_f = conv_b + n3_b @ conv_w

    return _prep(x, cls, color_centers, semantic_centers, a_embed, b_embed,
                 ce_w, ce_b, sem_w, q_w, qw_f, qb_f, fc1_f, fc2_w, conv_f,
                 c1_f, ccb_f, fc2_b, sem_b, n2_w, n2_b)


def _prep(x, cls, color_centers, semantic_centers, a_embed, b_embed,
          ce_w, ce_b, sem_w, q_w, qw_f, qb_f, fc1_f, fc2_w, conv_f,
          c1_f, ccb_f, fc2_b, sem_b, n2_w, n2_b):
    nz = lambda a: bool(np.any(a != 0))
    flags = {
        "qb": nz(qb_f),
        "semb": nz(sem_b),
        "ceb": nz(ce_b),
        "c1": nz(c1_f),
        "fc2b": nz(fc2_b),
        "ln2w": bool(np.any(n2_w != 1.0)),
        "ln2b": nz(n2_b),
        "ccb": nz(ccb_f),
    }

    xn = x.reshape(B, C, S)  # native [b, c, h*w] layout — already core-sharded
    in_maps = []
    for k in range(N_CORES):
        m = {
            "x": np.ascontiguousarray(xn[k]),
            "cls": np.ascontiguousarray(cls[k]),
            "cc_idx": color_centers,
            "centers": semantic_centers,
            "aemb": a_embed, "bemb": b_embed, "cew": ce_w,
            "semw": sem_w, "qw": qw_f,
            "fc1": fc1_f, "fc2": fc2_w, "conv": conv_f,
        }
        if flags["qb"]:
            m["qb"] = np.ascontiguousarray(qb_f[:, None])
        if flags["semb"]:
            m["semb"] = np.ascontiguousarray(sem_b[:, None])
        if flags["ceb"]:
            m["ceb"] = ce_b
        if flags["c1"]:
            m["c1b"] = np.ascontiguousarray(np.broadcast_to(c1_f, (P, D2)))
        if flags["fc2b"]:
            m["fc2b"] = np.ascontiguousarray(np.broadcast_to(fc2_b, (P, D2)))
        if flags["ln2w"]:
            m["ln2w"] = np.ascontiguousarray(np.broadcast_to(n2_w, (P, D2)))
        if flags["ln2b"]:
            m["ln2b"] = np.ascontiguousarray(np.broadcast_to(n2_b, (P, D2)))
        if flags["ccb"]:
            m["ccb"] = np.ascontiguousarray(ccb_f[:, None])
        in_maps.append(m)
    return flags, in_maps


def run(flags, in_maps, **kw):
    key = tuple(sorted(flags.items()))
    if key not in _CACHE:
        _CACHE[key] = build_bass(flags)
    nc = _CACHE[key]
    res = run_bass_kernel_spmd(nc, in_maps, core_ids=list(range(N_CORES)), **kw)
    out = np.stack([res.results[k]["out"] for k in range(N_CORES)], axis=0)
    return out.reshape(B, C, H, W), res


def kernel(**inputs):
    flags, in_maps = _prep_inputs(**inputs)
    out, _ = run(flags, in_maps)
    return out


def _prep_inputs(x, cls, color_centers, semantic_centers, a_embed, b_embed,
                 ce_w, ce_b, sem_w, sem_b, q_w, q_b,
                 n1_w, n1_b, n2_w, n2_b, n3_w, n3_b,
                 fc1_w, fc1_b, fc2_w, fc2_b, conv_w, conv_b):
    return _prep_inputs_impl(
        x, cls, color_centers, semantic_centers, a_embed, b_embed,
        ce_w, ce_b, sem_w, sem_b, q_w, q_b,
        n1_w, n1_b, n2_w, n2_b, n3_w, n3_b,
        fc1_w, fc1_b, fc2_w, fc2_b, conv_w, conv_b)



# revision 4
# speedup vs baseline: 1.5523x; 1.5523x over previous
"""Trainium2 Bass kernel for the ColorMemory block (v2).

Sharding: data-parallel over batch b across 8 NeuronCores (one batch element
per core); weights and the 512-row memory bank replicated per core.

All weight-only work is folded on the host:
  semP   = semantic_centers @ sem_w + sem_b                    [n, e]
  Wf     = (n1_w * q_w) @ semP^T                               [c, n]
  ncsum  = -sum_c Wf                                           [n]
  crow   = (n1_b @ q_w + q_b) @ semP^T                         [n]
  colemb = einsum('ind,bi->nd', host color-embed path, cls_b)  [n, ce]
  conv'  = n3_w * conv_w;  ccb = n3_b @ conv_w + conv_b

Device math per core (x stays feature-major [c, s]; LN1 is folded into the
logits as a rank-1 correction so the big matmul reads x straight from DRAM):
  l_raw[t,n] = x[:,t] @ Wf  +  mu1[t] * ncsum[n]  (+ sd1[t] * crow[n])
  p          = exp(rstd1 * l_raw - rstd1 * max)   denom via ACT accumulator
  cp         = (p^T)^T @ colemb / denom           [t, ce]
  y          = [x^T | cp], z2 = standardize(y)    (token-major, bf16)
  hT         = gelu(fc1^T @ z2T)                  (feature-major: no h transpose)
  mlp        = hT^T @ fc2, v = z2 + mlp
  outT       = conv'^T @ standardize(v)^T         [c, t] = native output layout

Three passes (exp -> gelu -> none) keep the ACT table resident; pass-3 rstd
uses a DVE Newton iteration so no third table load is needed.
"""

import numpy as np
import ml_dtypes
from contextlib import ExitStack

import concourse.bass as bass
import concourse.tile as tile
from concourse import bacc, mybir
from concourse.bass_utils import run_bass_kernel_spmd
from concourse.masks import make_identity

F32 = mybir.dt.float32
F32R = mybir.dt.float32r
BF16 = mybir.dt.bfloat16
I32 = mybir.dt.int32
AF = mybir.ActivationFunctionType
OP = mybir.AluOpType

N_CORES = 8
B, C, H, W = 8, 256, 64, 64
S = H * W              # 4096 tokens per core
NCOL = 512             # memory bank rows
CE = 256               # color embed dim
D2 = C + CE            # 512
EPS = 1e-5
P = 128

TW = 512               # token-tile width
NT = S // TW           # 8 tiles
NSUB = TW // P         # 4 subtiles per tile
CC = C // P            # 2 c-chunks
DC = D2 // P           # 4 chunks of the concat dim
NC_ = NCOL // P        # 4 n-chunks

RSQRT_MAGIC = 0x5F3759DF


def _newton_rstd(nc, pool, var_ap, ncols):
    """rstd [P, ncols] = (var+eps)^-0.5 via bit-magic + 2 Newton steps (DVE)."""
    a = pool.tile([P, ncols], F32, tag="nw_a")
    nc.vector.tensor_scalar(out=a[:], in0=var_ap, scalar1=float(EPS),
                            scalar2=None, op0=OP.add)
    tb = pool.tile([P, ncols], I32, tag="nw_b")
    nc.vector.tensor_scalar(out=tb[:], in0=a[:].bitcast(I32), scalar1=1,
                            scalar2=None, op0=OP.logical_shift_right)
    nb = pool.tile([P, ncols], I32, tag="nw_c")
    nc.vector.tensor_scalar(out=nb[:], in0=tb[:], scalar1=RSQRT_MAGIC,
                            scalar2=-1, op0=OP.subtract, op1=OP.mult)
    y = nb[:].bitcast(F32)
    y2 = None
    for _ in range(2):
        t = pool.tile([P, ncols], F32, tag="nw_t")
        nc.vector.tensor_tensor(out=t[:], in0=y, in1=y, op=OP.mult)
        nc.vector.tensor_tensor(out=t[:], in0=t[:], in1=a[:], op=OP.mult)
        nc.vector.tensor_scalar(out=t[:], in0=t[:], scalar1=-0.5,
                                scalar2=1.5, op0=OP.mult, op1=OP.add)
        y2 = pool.tile([P, ncols], F32, tag="nw_y")
        nc.vector.tensor_tensor(out=y2[:], in0=y, in1=t[:], op=OP.mult)
        y = y2[:]
    return y2


def build_bass(flags):
    nc = bacc.Bacc(
        "TRN2",
        target_bir_lowering=False,
        debug=False,
        enable_asserts=False,
        num_devices=N_CORES,
    )

    # ---- DRAM I/O (per-core shapes; weights pre-chunked on host) ----
    x_d = nc.dram_tensor("x", [C, S], F32R, kind="ExternalInput").ap()
    wf_d = nc.dram_tensor("wf", [P, CC, NCOL], F32R, kind="ExternalInput").ap()
    ncsum_d = nc.dram_tensor("ncsum", [1, NCOL], BF16, kind="ExternalInput").ap()
    colemb_d = nc.dram_tensor("colemb", [P, NC_, CE], BF16, kind="ExternalInput").ap()
    fc1_d = nc.dram_tensor("fc1", [P, DC, D2], BF16, kind="ExternalInput").ap()
    fc2_d = nc.dram_tensor("fc2", [P, DC, D2], BF16, kind="ExternalInput").ap()
    conv_d = nc.dram_tensor("conv", [P, DC, C], BF16, kind="ExternalInput").ap()
    opt = {}
    if flags["qcr"]:
        opt["crow"] = nc.dram_tensor("crow", [1, NCOL], BF16, kind="ExternalInput").ap()
    if flags["c1"]:
        opt["c1"] = nc.dram_tensor("c1b", [P, DC], F32, kind="ExternalInput").ap()
    if flags["fc2b"]:
        opt["fc2b"] = nc.dram_tensor("fc2b", [1, D2], BF16, kind="ExternalInput").ap()
    if flags["ln2w"]:
        opt["ln2w"] = nc.dram_tensor("ln2w", [P, D2], F32, kind="ExternalInput").ap()
    if flags["ln2b"]:
        opt["ln2b"] = nc.dram_tensor("ln2b", [P, D2], F32, kind="ExternalInput").ap()
    if flags["ccb"]:
        opt["ccb"] = nc.dram_tensor("ccb", [P, CC], F32, kind="ExternalInput").ap()
    out_d = nc.dram_tensor("out", [C, S], F32, kind="ExternalOutput").ap()

    with tile.TileContext(nc) as tc, ExitStack() as ctx:
        wpool = ctx.enter_context(tc.tile_pool(name="weights", bufs=1))
        z2pool = ctx.enter_context(tc.tile_pool(name="z2store", bufs=NT * NSUB))

        ident_f32 = wpool.tile([P, P], F32)
        make_identity(nc, ident_f32[:])
        ident_r = wpool.tile([P, P], F32R)
        nc.vector.tensor_copy(out=ident_r[:], in_=ident_f32[:])
        ident_b = wpool.tile([P, P], BF16)
        nc.vector.tensor_copy(out=ident_b[:], in_=ident_f32[:])
        eps_col = wpool.tile([P, 1], F32)
        nc.vector.memset(eps_col[:], EPS)

        wf_sb = wpool.tile([P, CC, NCOL], F32R)
        nc.sync.dma_start(out=wf_sb[:], in_=wf_d)
        ncsum_sb = wpool.tile([1, NCOL], BF16)
        nc.sync.dma_start(out=ncsum_sb[:], in_=ncsum_d)
        colemb_sb = wpool.tile([P, NC_, CE], BF16)
        nc.sync.dma_start(out=colemb_sb[:], in_=colemb_d)
        fc1_sb = wpool.tile([P, DC, D2], BF16)
        nc.sync.dma_start(out=fc1_sb[:], in_=fc1_d)
        fc2_sb = wpool.tile([P, DC, D2], BF16)
        nc.sync.dma_start(out=fc2_sb[:], in_=fc2_d)
        conv_sb = wpool.tile([P, DC, C], BF16)
        nc.sync.dma_start(out=conv_sb[:], in_=conv_d)

        bias_sb = {}
        for key, shape, dt in (("crow", [1, NCOL], BF16), ("c1", [P, DC], F32),
                               ("fc2b", [1, D2], BF16), ("ln2w", [P, D2], F32),
                               ("ln2b", [P, D2], F32), ("ccb", [P, CC], F32)):
            if key in opt:
                t = wpool.tile(shape, dt, name=f"b_{key}")
                nc.sync.dma_start(out=t[:], in_=opt[key])
                bias_sb[key] = t
        ones_bf = None
        if flags["fc2b"]:
            ones_bf = wpool.tile([1, P], BF16)
            nc.vector.memset(ones_bf[:], 1.0)

        z2_tiles = []

        # ================= pass 1: attention =================
        with (
            tc.tile_pool(name="p1x", bufs=3) as xpool,
            tc.tile_pool(name="p1y", bufs=6) as ypool,
            tc.tile_pool(name="p1p", bufs=3) as ppool,
            tc.tile_pool(name="p1pt", bufs=3) as ptpool,
            tc.tile_pool(name="p1row", bufs=4) as rowpool,
            tc.tile_pool(name="p1stats", bufs=8) as stats,
            tc.tile_pool(name="p1tpx", bufs=2, space="PSUM") as ps_xt,
            tc.tile_pool(name="p1row_ps", bufs=1, space="PSUM") as ps_row,
            tc.tile_pool(name="p1l", bufs=2, space="PSUM") as ps_l,
            tc.tile_pool(name="p1pt_ps", bufs=2, space="PSUM") as ps_pt,
            tc.tile_pool(name="p1cp", bufs=1, space="PSUM") as ps_cp,
        ):
            for T in range(NT):
                x_t = xpool.tile([P, CC, TW], F32R, tag="x")
                for cc in range(CC):
                    nc.sync.dma_start(
                        out=x_t[:, cc, :],
                        in_=x_d[cc * P:(cc + 1) * P, T * TW:(T + 1) * TW],
                    )
                for s in range(NSUB):
                    # x^T into the concat tile (token-major)
                    y_t = ypool.tile([P, D2], BF16, tag="y")
                    tpx = ps_xt.tile([P, CC, P], F32R, tag="tpx")
                    for cc in range(CC):
                        nc.tensor.transpose(
                            out=tpx[:, cc, :],
                            in_=x_t[:, cc, s * P:(s + 1) * P],
                            identity=ident_r[:],
                        )
                    nc.any.tensor_copy(out=y_t[:, 0:C], in_=tpx[:])
                    # LN1 stats (token axis)
                    st1 = stats.tile([P, nc.vector.BN_STATS_DIM], F32, tag="st1")
                    nc.vector.bn_stats(out=st1[:], in_=y_t[:, 0:C])
                    mv1 = stats.tile([P, 2], F32, tag="mv1")
                    nc.vector.bn_aggr(out=mv1[:], in_=st1[:])
                    lnv = stats.tile([P, 1], F32, tag="lnv")
                    nc.scalar.activation(out=lnv[:], in_=mv1[:, 1:2], func=AF.Ln,
                                         bias=eps_col[:])
                    rstd1 = stats.tile([P, 1], F32, tag="rstd1")
                    nc.scalar.activation(out=rstd1[:], in_=lnv[:], func=AF.Exp,
                                         scale=-0.5)
                    # mu1 (and sd1 if needed) as bf16 rows for the rank-1 fixups
                    mub = stats.tile([P, 1], BF16, tag="mub")
                    nc.vector.tensor_copy(out=mub[:], in_=mv1[:, 0:1])
                    rowp = ps_row.tile([1, P], F32, tag="rowp")
                    nc.tensor.matmul(out=rowp[:], lhsT=mub[:], rhs=ident_b[:],
                                     start=True, stop=True)
                    murow = rowpool.tile([1, P], BF16, tag="murow")
                    nc.scalar.copy(out=murow[:], in_=rowp[:])
                    if flags["qcr"]:
                        sd = stats.tile([P, 1], F32, tag="sd")
                        nc.scalar.activation(out=sd[:], in_=lnv[:], func=AF.Exp,
                                             scale=0.5)
                        sdb = stats.tile([P, 1], BF16, tag="sdb")
                        nc.vector.tensor_copy(out=sdb[:], in_=sd[:])
                        rowp2 = ps_row.tile([1, P], F32, tag="rowp2")
                        nc.tensor.matmul(out=rowp2[:], lhsT=sdb[:], rhs=ident_b[:],
                                         start=True, stop=True)
                        sdrow = rowpool.tile([1, P], BF16, tag="sdrow")
                        nc.scalar.copy(out=sdrow[:], in_=rowp2[:])
                    # logits: x @ Wf + mu1 (x) ncsum (+ sd1 (x) crow)
                    psl = ps_l.tile([P, NCOL], F32, tag="l")
                    nc.tensor.matmul(out=psl[:], lhsT=x_t[:, 0, s * P:(s + 1) * P],
                                     rhs=wf_sb[:, 0, :], start=True, stop=False)
                    nc.tensor.matmul(out=psl[:], lhsT=x_t[:, 1, s * P:(s + 1) * P],
                                     rhs=wf_sb[:, 1, :], start=False, stop=False)
                    nc.tensor.matmul(out=psl[:], lhsT=murow[:], rhs=ncsum_sb[:],
                                     start=False, stop=not flags["qcr"])
                    if flags["qcr"]:
                        nc.tensor.matmul(out=psl[:], lhsT=sdrow[:],
                                         rhs=bias_sb["crow"][:],
                                         start=False, stop=True)
                    # softmax: p = exp(rstd1*l - rstd1*max), denom accumulated
                    negmax = stats.tile([P, 1], F32, tag="negmax")
                    nc.vector.reduce_max(out=negmax[:], in_=psl[:],
                                         axis=mybir.AxisListType.X, negate=True)
                    nms = stats.tile([P, 1], F32, tag="nms")
                    nc.vector.tensor_tensor(out=nms[:], in0=negmax[:], in1=rstd1[:],
                                            op=OP.mult)
                    p_sb = ppool.tile([P, NCOL], BF16, tag="p")
                    denom = stats.tile([P, 1], F32, tag="denom")
                    nc.scalar.activation(out=p_sb[:], in_=psl[:], func=AF.Exp,
                                         bias=nms[:], scale=rstd1[:],
                                         accum_out=denom[:])
                    recip = stats.tile([P, 1], F32, tag="recip")
                    nc.vector.reciprocal(out=recip[:], in_=denom[:])
                    # p^T, then color prior straight into the concat tile
                    ptp = ps_pt.tile([P, NC_, P], BF16, tag="ptp")
                    for j in range(NC_):
                        nc.tensor.transpose(out=ptp[:, j, :],
                                            in_=p_sb[:, j * P:(j + 1) * P],
                                            identity=ident_b[:])
                    pt_sb = ptpool.tile([P, NC_, P], BF16, tag="pt")
                    nc.scalar.copy(out=pt_sb[:], in_=ptp[:])
                    pcp = ps_cp.tile([P, CE], F32, tag="cp")
                    for j in range(NC_):
                        nc.tensor.matmul(out=pcp[:], lhsT=pt_sb[:, j, :],
                                         rhs=colemb_sb[:, j, :],
                                         start=(j == 0), stop=(j == NC_ - 1))
                    nc.vector.tensor_scalar(out=y_t[:, C:D2], in0=pcp[:],
                                            scalar1=recip[:], scalar2=None,
                                            op0=OP.mult)
                    # LN2 -> z2 (kept in SBUF for passes 2/3)
                    st2 = stats.tile([P, nc.vector.BN_STATS_DIM], F32, tag="st2")
                    nc.vector.bn_stats(out=st2[:], in_=y_t[:])
                    mv2 = stats.tile([P, 2], F32, tag="mv2")
                    nc.vector.bn_aggr(out=mv2[:], in_=st2[:])
                    lnv2 = stats.tile([P, 1], F32, tag="lnv2")
                    nc.scalar.activation(out=lnv2[:], in_=mv2[:, 1:2], func=AF.Ln,
                                         bias=eps_col[:])
                    rstd2 = stats.tile([P, 1], F32, tag="rstd2")
                    nc.scalar.activation(out=rstd2[:], in_=lnv2[:], func=AF.Exp,
                                         scale=-0.5)
                    z2_t = z2pool.tile([P, D2], BF16, tag="z2")
                    nc.vector.tensor_scalar(out=z2_t[:], in0=y_t[:],
                                            scalar1=mv2[:, 0:1], scalar2=rstd2[:],
                                            op0=OP.subtract, op1=OP.mult)
                    if flags["ln2w"]:
                        nc.vector.tensor_tensor(out=z2_t[:], in0=z2_t[:],
                                                in1=bias_sb["ln2w"][:], op=OP.mult)
                    if flags["ln2b"]:
                        nc.vector.tensor_tensor(out=z2_t[:], in0=z2_t[:],
                                                in1=bias_sb["ln2b"][:], op=OP.add)
                    z2_tiles.append(z2_t)

        tc.no_sync_barrier()

        # ================= pass 2: MLP (gelu) =================
        with (
            tc.tile_pool(name="p2z2T", bufs=2) as z2Tpool,
            tc.tile_pool(name="p2hT", bufs=2) as hTpool,
            tc.tile_pool(name="p2tz", bufs=2, space="PSUM") as ps_tz,
            tc.tile_pool(name="p2h", bufs=3, space="PSUM") as ps_h,
            tc.tile_pool(name="p2m", bufs=2, space="PSUM") as ps_m,
        ):
            for T in range(NT):
                z2T = z2Tpool.tile([P, DC, TW], BF16, tag="z2T")
                for f in range(DC):
                    tz = ps_tz.tile([P, TW], BF16, tag="tz")
                    for s in range(NSUB):
                        nc.tensor.transpose(
                            out=tz[:, s * P:(s + 1) * P],
                            in_=z2_tiles[T * NSUB + s][:, f * P:(f + 1) * P],
                            identity=ident_b[:])
                    nc.any.tensor_copy(out=z2T[:, f, :], in_=tz[:])
                # hT = gelu(fc1^T @ z2T): feature-major, no transpose of h
                hT = hTpool.tile([P, DC, TW], BF16, tag="hT")
                for o in range(DC):
                    ph = ps_h.tile([P, TW], F32, tag="h")
                    for f in range(DC):
                        nc.tensor.matmul(out=ph[:],
                                         lhsT=fc1_sb[:, f, o * P:(o + 1) * P],
                                         rhs=z2T[:, f, :],
                                         start=(f == 0), stop=(f == DC - 1))
                    bias = bias_sb["c1"][:, o:o + 1] if flags["c1"] else 0.0
                    nc.scalar.activation(out=hT[:, o, :], in_=ph[:], func=AF.Gelu,
                                         bias=bias)
                # mlp (token-major) + residual, v overwrites the z2 slot
                for s in range(NSUB):
                    pm = ps_m.tile([P, TW], F32, tag="m")
                    for o in range(DC):
                        nc.tensor.matmul(out=pm[:],
                                         lhsT=hT[:, o, s * P:(s + 1) * P],
                                         rhs=fc2_sb[:, o, :], start=(o == 0),
                                         stop=(o == DC - 1) and not flags["fc2b"])
                    if flags["fc2b"]:
                        nc.tensor.matmul(out=pm[:], lhsT=ones_bf[:],
                                         rhs=bias_sb["fc2b"][:],
                                         start=False, stop=True)
                    z2_t = z2_tiles[T * NSUB + s]
                    nc.vector.tensor_tensor(out=z2_t[:], in0=z2_t[:], in1=pm[:],
                                            op=OP.add)

        tc.no_sync_barrier()

        # ================= pass 3: LN3 + output conv =================
        with (
            tc.tile_pool(name="p3z3", bufs=6) as z3pool,
            tc.tile_pool(name="p3z3T", bufs=2) as z3Tpool,
            tc.tile_pool(name="p3o", bufs=4) as opool,
            tc.tile_pool(name="p3stats", bufs=4) as stats3,
            tc.tile_pool(name="p3tz", bufs=2, space="PSUM") as ps_tz3,
            tc.tile_pool(name="p3o_ps", bufs=2, space="PSUM") as ps_o,
        ):
            for T in range(NT):
                mv3 = stats3.tile([P, NSUB, 2], F32, tag="mv3")
                for s in range(NSUB):
                    st3 = stats3.tile([P, nc.vector.BN_STATS_DIM], F32, tag="st3")
                    nc.vector.bn_stats(out=st3[:], in_=z2_tiles[T * NSUB + s][:])
                    nc.vector.bn_aggr(out=mv3[:, s, :], in_=st3[:])
                rstd3 = _newton_rstd(nc, stats3, mv3[:, :, 1], NSUB)
                z3_list = []
                for s in range(NSUB):
                    z3_t = z3pool.tile([P, D2], BF16, tag="z3")
                    nc.vector.tensor_scalar(out=z3_t[:],
                                            in0=z2_tiles[T * NSUB + s][:],
                                            scalar1=mv3[:, s, 0:1],
                                            scalar2=rstd3[:, s:s + 1],
                                            op0=OP.subtract, op1=OP.mult)
                    z3_list.append(z3_t)
                z3T = z3Tpool.tile([P, DC, TW], BF16, tag="z3T")
                for f in range(DC):
                    tz = ps_tz3.tile([P, TW], BF16, tag="tz3")
                    for s in range(NSUB):
                        nc.tensor.transpose(
                            out=tz[:, s * P:(s + 1) * P],
                            in_=z3_list[s][:, f * P:(f + 1) * P],
                            identity=ident_b[:])
                    nc.any.tensor_copy(out=z3T[:, f, :], in_=tz[:])
                for cc in range(CC):
                    po = ps_o.tile([P, TW], F32, tag="o")
                    for f in range(DC):
                        nc.tensor.matmul(out=po[:],
                                         lhsT=conv_sb[:, f, cc * P:(cc + 1) * P],
                                         rhs=z3T[:, f, :],
                                         start=(f == 0), stop=(f == DC - 1))
                    ot = opool.tile([P, TW], F32, tag="ot")
                    if flags["ccb"]:
                        nc.any.tensor_scalar(out=ot[:], in0=po[:],
                                             scalar1=bias_sb["ccb"][:, cc:cc + 1],
                                             scalar2=None, op0=OP.add)
                    else:
                        nc.any.tensor_copy(out=ot[:], in_=po[:])
                    nc.sync.dma_start(
                        out=out_d[cc * P:(cc + 1) * P, T * TW:(T + 1) * TW],
                        in_=ot[:])

    nc.compile()
    return nc


_CACHE = {}


def _chunk(a, p=P):
    """[K, N] -> [P, K//P, N] (k-chunks on partitions)."""
    k, n = a.shape
    return np.ascontiguousarray(a.reshape(k // p, p, n).transpose(1, 0, 2))


def _prep_inputs_impl(x, cls, color_centers, semantic_centers, a_embed, b_embed,
                      ce_w, ce_b, sem_w, sem_b, q_w, q_b,
                      n1_w, n1_b, n2_w, n2_b, n3_w, n3_b,
                      fc1_w, fc1_b, fc2_w, fc2_b, conv_w, conv_b):
    f32 = lambda a: np.asarray(a, np.float32)
    bf = lambda a: np.ascontiguousarray(np.asarray(a, ml_dtypes.bfloat16))
    x = np.ascontiguousarray(f32(x))
    cls = f32(cls)
    color_centers = np.asarray(color_centers, np.int64)
    semantic_centers = f32(semantic_centers)
    a_embed, b_embed = f32(a_embed), f32(b_embed)
    ce_w, ce_b = f32(ce_w), f32(ce_b)
    sem_w, sem_b = f32(sem_w), f32(sem_b)
    q_w, q_b = f32(q_w), f32(q_b)
    n1_w, n1_b = f32(n1_w), f32(n1_b)
    n2_w, n2_b = f32(n2_w), f32(n2_b)
    n3_w, n3_b = f32(n3_w), f32(n3_b)
    fc1_w, fc1_b = f32(fc1_w), f32(fc1_b)
    fc2_w, fc2_b = f32(fc2_w), f32(fc2_b)
    conv_w, conv_b = f32(conv_w), f32(conv_b)

    # ---- host-side weight folding ----
    semP = semantic_centers @ sem_w + sem_b                  # [n, e]
    Wf = (n1_w[:, None] * q_w) @ semP.T                      # [c, n]
    ncsum = -Wf.sum(0)                                       # [n]
    crow = (n1_b @ q_w + q_b) @ semP.T                       # [n]
    ab = np.concatenate([a_embed[color_centers[:, :, 0]],
                         b_embed[color_centers[:, :, 1]]], -1)   # [4, n, 2ce]
    ce = np.einsum('inf,ifd->ind', ab, ce_w) + ce_b[:, None, :]  # [4, n, ce]
    colemb_all = np.einsum('ind,bi->bnd', ce, cls)               # [b, n, ce]
    conv_f = n3_w[:, None] * conv_w
    ccb = n3_b @ conv_w + conv_b

    nz = lambda a: bool(np.any(a != 0))
    flags = {
        "qcr": nz(crow),
        "c1": nz(fc1_b),
        "fc2b": nz(fc2_b),
        "ln2w": bool(np.any(n2_w != 1.0)),
        "ln2b": nz(n2_b),
        "ccb": nz(ccb),
    }

    wf_p = _chunk(Wf)                                        # [P, CC, NCOL] f32
    fc1_p = bf(_chunk(fc1_w))                                # [P, DC, D2]
    fc2_p = bf(_chunk(fc2_w))
    conv_p = bf(_chunk(conv_f))                              # [P, DC, C]
    ncsum_p = bf(ncsum[None, :])

    xn = x.reshape(B, C, S)
    in_maps = []
    for k in range(N_CORES):
        m = {
            "x": np.ascontiguousarray(xn[k]),
            "wf": wf_p,
            "ncsum": ncsum_p,
            "colemb": bf(_chunk(colemb_all[k])),             # [P, NC_, CE]
            "fc1": fc1_p,
            "fc2": fc2_p,
            "conv": conv_p,
        }
        if flags["qcr"]:
            m["crow"] = bf(crow[None, :])
        if flags["c1"]:
            m["c1b"] = np.ascontiguousarray(fc1_b.reshape(DC, P).T)
        if flags["fc2b"]:
            m["fc2b"] = bf(fc2_b[None, :])
        if flags["ln2w"]:
            m["ln2w"] = np.ascontiguousarray(np.broadcast_to(n2_w, (P, D2)))
        if flags["ln2b"]:
            m["ln2b"] = np.ascontiguousarray(np.broadcast_to(n2_b, (P, D2)))
        if flags["ccb"]:
            m["ccb"] = np.ascontiguousarray(ccb.reshape(CC, P).T)
        in_maps.append(m)
    return flags, in_maps


def run(flags, in_maps, **kw):
    key = tuple(sorted(flags.items()))
    if key not in _CACHE:
        _CACHE[key] = build_bass(flags)
    nc = _CACHE[key]
    res = run_bass_kernel_spmd(nc, in_maps, core_ids=list(range(N_CORES)), **kw)
    out = np.stack([res.results[k]["out"] for k in range(N_CORES)], axis=0)
    return out.reshape(B, C, H, W), res


def kernel(**inputs):
    flags, in_maps = _prep_inputs(**inputs)
    out, _ = run(flags, in_maps)
    return out


def _prep_inputs(x, cls, color_centers, semantic_centers, a_embed, b_embed,
                 ce_w, ce_b, sem_w, sem_b, q_w, q_b,
                 n1_w, n1_b, n2_w, n2_b, n3_w, n3_b,
                 fc1_w, fc1_b, fc2_w, fc2_b, conv_w, conv_b):
    return _prep_inputs_impl(
        x, cls, color_centers, semantic_centers, a_embed, b_embed,
        ce_w, ce_b, sem_w, sem_b, q_w, q_b,
        n1_w, n1_b, n2_w, n2_b, n3_w, n3_b,
        fc1_w, fc1_b, fc2_w, fc2_b, conv_w, conv_b)


# revision 6
# speedup vs baseline: 1.8178x; 1.1710x over previous
"""Trainium2 Bass kernel for the ColorMemory block (v3).

Sharding: data-parallel over batch b across 8 NeuronCores (one batch element
per core); weights and the 512-row memory bank replicated per core.

All weight-only work is folded on the host:
  semP   = semantic_centers @ sem_w + sem_b                    [n, e]
  Wf     = (n1_w * q_w) @ semP^T                               [c, n]
  ncsum  = -sum_c Wf                                           [n]
  crow   = (n1_b @ q_w + q_b) @ semP^T                         [n]
  colemb = einsum('ind,bi->nd', host color-embed path, cls_b)  [n, ce]
  conv'  = n3_w * conv_w;  ccb = n3_b @ conv_w + conv_b

Device math per core (x stays feature-major [c, s]; LN1 is folded into the
logits as a rank-1 correction so the big matmul reads x straight from DRAM):
  l_raw[t,n] = x[:,t] @ Wf  +  mu1[t] * ncsum[n]  (+ sd1[t] * crow[n])
  p          = exp(rstd1 * l_raw - 96)            (no per-token max: softmax is
               shift-invariant and |logit| <= ~150 << the f32 exp range)
  cp         = (p^T)^T @ colemb / denom           [t, ce]
  y          = [x^T | cp], z2 = standardize(y)    (token-major, bf16)
  hT         = gelu(fc1^T @ z2T)                  (feature-major: no h transpose)
  mlp        = hT^T @ fc2, v = z2 + mlp
  outT       = conv'^T @ standardize(v)^T         [c, t] = native output layout

ACT table discipline: pass 1 uses only {Exp, Identity, Copy} (one table set),
pass 2 only {Gelu, Copy}; rstd everywhere is a DVE Newton iteration, so only
two table loads happen in the whole kernel.
"""

import numpy as np
import ml_dtypes
from contextlib import ExitStack

import concourse.bass as bass
import concourse.tile as tile
from concourse import bacc, mybir
from concourse.bass_utils import run_bass_kernel_spmd
from concourse.masks import make_identity

F32 = mybir.dt.float32
F32R = mybir.dt.float32r
BF16 = mybir.dt.bfloat16
I32 = mybir.dt.int32
AF = mybir.ActivationFunctionType
OP = mybir.AluOpType

N_CORES = 8
B, C, H, W = 8, 256, 64, 64
S = H * W              # 4096 tokens per core
NCOL = 512             # memory bank rows
CE = 256               # color embed dim
D2 = C + CE            # 512
EPS = 1e-5
P = 128

TW = 512               # token-tile width
NT = S // TW           # 8 tiles
NSUB = TW // P         # 4 subtiles per tile
CC = C // P            # 2 c-chunks
DC = D2 // P           # 4 chunks of the concat dim
NC_ = NCOL // P        # 4 n-chunks

LOGIT_SHIFT = -96.0    # replaces per-token max subtraction (see module doc)
RSQRT_MAGIC = 0x5F3759DF


def _newton_rstd(nc, pool, var_ap, ncols):
    """rstd [P, ncols] = (var+eps)^-0.5 via bit-magic + 2 Newton steps (DVE)."""
    a = pool.tile([P, ncols], F32, tag="nw_a")
    nc.vector.tensor_scalar(out=a[:], in0=var_ap, scalar1=float(EPS),
                            scalar2=None, op0=OP.add)
    tb = pool.tile([P, ncols], I32, tag="nw_b")
    nc.vector.tensor_scalar(out=tb[:], in0=a[:].bitcast(I32), scalar1=1,
                            scalar2=None, op0=OP.logical_shift_right)
    nb = pool.tile([P, ncols], I32, tag="nw_c")
    nc.vector.tensor_scalar(out=nb[:], in0=tb[:], scalar1=RSQRT_MAGIC,
                            scalar2=-1, op0=OP.subtract, op1=OP.mult)
    y = nb[:].bitcast(F32)
    y2 = None
    for _ in range(2):
        t = pool.tile([P, ncols], F32, tag="nw_t")
        nc.vector.tensor_tensor(out=t[:], in0=y, in1=y, op=OP.mult)
        nc.vector.tensor_tensor(out=t[:], in0=t[:], in1=a[:], op=OP.mult)
        nc.vector.tensor_scalar(out=t[:], in0=t[:], scalar1=-0.5,
                                scalar2=1.5, op0=OP.mult, op1=OP.add)
        y2 = pool.tile([P, ncols], F32, tag="nw_y")
        nc.vector.tensor_tensor(out=y2[:], in0=y, in1=t[:], op=OP.mult)
        y = y2[:]
    return y2


def build_bass(flags):
    nc = bacc.Bacc(
        "TRN2",
        target_bir_lowering=False,
        debug=False,
        enable_asserts=False,
        num_devices=N_CORES,
    )

    # ---- DRAM I/O (per-core shapes; weights pre-chunked on host) ----
    x_d = nc.dram_tensor("x", [C, S], F32R, kind="ExternalInput").ap()
    wf_d = nc.dram_tensor("wf", [P, CC, NCOL], F32R, kind="ExternalInput").ap()
    ncsum_d = nc.dram_tensor("ncsum", [1, NCOL], BF16, kind="ExternalInput").ap()
    colemb_d = nc.dram_tensor("colemb", [P, NC_, CE], BF16, kind="ExternalInput").ap()
    fc1_d = nc.dram_tensor("fc1", [P, DC, D2], BF16, kind="ExternalInput").ap()
    fc2_d = nc.dram_tensor("fc2", [P, DC, D2], BF16, kind="ExternalInput").ap()
    conv_d = nc.dram_tensor("conv", [P, DC, C], BF16, kind="ExternalInput").ap()
    opt = {}
    if flags["qcr"]:
        opt["crow"] = nc.dram_tensor("crow", [1, NCOL], BF16, kind="ExternalInput").ap()
    if flags["c1"]:
        opt["c1"] = nc.dram_tensor("c1b", [P, DC], F32, kind="ExternalInput").ap()
    if flags["fc2b"]:
        opt["fc2b"] = nc.dram_tensor("fc2b", [1, D2], BF16, kind="ExternalInput").ap()
    if flags["ln2w"]:
        opt["ln2w"] = nc.dram_tensor("ln2w", [P, D2], F32, kind="ExternalInput").ap()
    if flags["ln2b"]:
        opt["ln2b"] = nc.dram_tensor("ln2b", [P, D2], F32, kind="ExternalInput").ap()
    if flags["ccb"]:
        opt["ccb"] = nc.dram_tensor("ccb", [P, CC], F32, kind="ExternalInput").ap()
    out_d = nc.dram_tensor("out", [C, S], F32, kind="ExternalOutput").ap()

    with tile.TileContext(nc) as tc, ExitStack() as ctx:
        wpool = ctx.enter_context(tc.tile_pool(name="weights", bufs=1))
        z2pool = ctx.enter_context(tc.tile_pool(name="z2store", bufs=NT * NSUB))

        ident_f32 = wpool.tile([P, P], F32)
        make_identity(nc, ident_f32[:])
        ident_r = wpool.tile([P, P], F32R)
        nc.vector.tensor_copy(out=ident_r[:], in_=ident_f32[:])
        ident_b = wpool.tile([P, P], BF16)
        nc.vector.tensor_copy(out=ident_b[:], in_=ident_f32[:])

        shift_col = wpool.tile([P, 1], F32)
        nc.vector.memset(shift_col[:], LOGIT_SHIFT)

        wf_sb = wpool.tile([P, CC, NCOL], F32R)
        nc.sync.dma_start(out=wf_sb[:], in_=wf_d)
        ncsum_sb = wpool.tile([1, NCOL], BF16)
        nc.sync.dma_start(out=ncsum_sb[:], in_=ncsum_d)
        colemb_sb = wpool.tile([P, NC_, CE], BF16)
        nc.sync.dma_start(out=colemb_sb[:], in_=colemb_d)
        fc1_sb = wpool.tile([P, DC, D2], BF16)
        nc.sync.dma_start(out=fc1_sb[:], in_=fc1_d)
        fc2_sb = wpool.tile([P, DC, D2], BF16)
        nc.sync.dma_start(out=fc2_sb[:], in_=fc2_d)
        conv_sb = wpool.tile([P, DC, C], BF16)
        nc.sync.dma_start(out=conv_sb[:], in_=conv_d)

        bias_sb = {}
        for key, shape, dt in (("crow", [1, NCOL], BF16), ("c1", [P, DC], F32),
                               ("fc2b", [1, D2], BF16), ("ln2w", [P, D2], F32),
                               ("ln2b", [P, D2], F32), ("ccb", [P, CC], F32)):
            if key in opt:
                t = wpool.tile(shape, dt, name=f"b_{key}")
                nc.sync.dma_start(out=t[:], in_=opt[key])
                bias_sb[key] = t
        ones_bf = None
        if flags["fc2b"]:
            ones_bf = wpool.tile([1, P], BF16)
            nc.vector.memset(ones_bf[:], 1.0)

        z2_tiles = []

        # ================= pass 1: attention =================
        with (
            tc.tile_pool(name="p1x", bufs=3) as xpool,
            tc.tile_pool(name="p1y", bufs=6) as ypool,
            tc.tile_pool(name="p1p", bufs=3) as ppool,
            tc.tile_pool(name="p1pt", bufs=3) as ptpool,
            tc.tile_pool(name="p1row", bufs=4) as rowpool,
            tc.tile_pool(name="p1stats", bufs=8) as stats,
            tc.tile_pool(name="p1tpx", bufs=2, space="PSUM") as ps_xt,
            tc.tile_pool(name="p1row_ps", bufs=1, space="PSUM") as ps_row,
            tc.tile_pool(name="p1l", bufs=2, space="PSUM") as ps_l,
            tc.tile_pool(name="p1pt_ps", bufs=2, space="PSUM") as ps_pt,
            tc.tile_pool(name="p1cp", bufs=1, space="PSUM") as ps_cp,
        ):
            for T in range(NT):
                x_t = xpool.tile([P, CC, TW], F32R, tag="x")
                for cc in range(CC):
                    nc.sync.dma_start(
                        out=x_t[:, cc, :],
                        in_=x_d[cc * P:(cc + 1) * P, T * TW:(T + 1) * TW],
                    )
                for pp in range(NSUB // 2):
                    # ---- stage A: x^T + LN1 stats for the pair ----
                    y_pair = []
                    mv1p = stats.tile([P, 2, 2], F32, tag="mv1p")
                    for h in range(2):
                        s = 2 * pp + h
                        y_t = ypool.tile([P, D2], BF16, tag="y")
                        tpx = ps_xt.tile([P, CC, P], F32R, tag="tpx")
                        for cc in range(CC):
                            nc.tensor.transpose(
                                out=tpx[:, cc, :],
                                in_=x_t[:, cc, s * P:(s + 1) * P],
                                identity=ident_r[:],
                            )
                        nc.any.tensor_copy(out=y_t[:, 0:C], in_=tpx[:])
                        st1 = stats.tile([P, nc.vector.BN_STATS_DIM], F32, tag="st1")
                        nc.vector.bn_stats(out=st1[:], in_=y_t[:, 0:C])
                        nc.vector.bn_aggr(out=mv1p[:, h, :], in_=st1[:])
                        y_pair.append(y_t)
                    rstd1p = _newton_rstd(nc, stats, mv1p[:, :, 1], 2)
                    # ---- stage B: logits + softmax + color prior ----
                    mv2p = stats.tile([P, 2, 2], F32, tag="mv2p")
                    for h in range(2):
                        s = 2 * pp + h
                        y_t = y_pair[h]
                        mub = stats.tile([P, 1], BF16, tag="mub")
                        nc.scalar.activation(out=mub[:], in_=mv1p[:, h, 0:1],
                                             func=AF.Identity)
                        rowp = ps_row.tile([1, P], F32, tag="rowp")
                        nc.tensor.matmul(out=rowp[:], lhsT=mub[:], rhs=ident_b[:],
                                         start=True, stop=True)
                        murow = rowpool.tile([1, P], BF16, tag="murow")
                        nc.scalar.copy(out=murow[:], in_=rowp[:])
                        if flags["qcr"]:
                            sd = stats.tile([P, 1], F32, tag="sd")
                            nc.vector.reciprocal(out=sd[:],
                                                 in_=rstd1p[:, h:h + 1])
                            sdb = stats.tile([P, 1], BF16, tag="sdb")
                            nc.scalar.activation(out=sdb[:], in_=sd[:],
                                                 func=AF.Identity)
                            rowp2 = ps_row.tile([1, P], F32, tag="rowp2")
                            nc.tensor.matmul(out=rowp2[:], lhsT=sdb[:],
                                             rhs=ident_b[:], start=True, stop=True)
                            sdrow = rowpool.tile([1, P], BF16, tag="sdrow")
                            nc.scalar.copy(out=sdrow[:], in_=rowp2[:])
                        psl = ps_l.tile([P, NCOL], F32, tag="l")
                        nc.tensor.matmul(out=psl[:],
                                         lhsT=x_t[:, 0, s * P:(s + 1) * P],
                                         rhs=wf_sb[:, 0, :], start=True, stop=False)
                        nc.tensor.matmul(out=psl[:],
                                         lhsT=x_t[:, 1, s * P:(s + 1) * P],
                                         rhs=wf_sb[:, 1, :], start=False, stop=False)
                        nc.tensor.matmul(out=psl[:], lhsT=murow[:], rhs=ncsum_sb[:],
                                         start=False, stop=not flags["qcr"])
                        if flags["qcr"]:
                            nc.tensor.matmul(out=psl[:], lhsT=sdrow[:],
                                             rhs=bias_sb["crow"][:],
                                             start=False, stop=True)
                        p_sb = ppool.tile([P, NCOL], BF16, tag="p")
                        denom = stats.tile([P, 1], F32, tag="denom")
                        nc.scalar.activation(out=p_sb[:], in_=psl[:], func=AF.Exp,
                                             bias=shift_col[:],
                                             scale=rstd1p[:, h:h + 1],
                                             accum_out=denom[:])
                        recip = stats.tile([P, 1], F32, tag="recip")
                        nc.vector.reciprocal(out=recip[:], in_=denom[:])
                        ptp = ps_pt.tile([P, NC_, P], BF16, tag="ptp")
                        for j in range(NC_):
                            nc.tensor.transpose(out=ptp[:, j, :],
                                                in_=p_sb[:, j * P:(j + 1) * P],
                                                identity=ident_b[:])
                        pt_sb = ptpool.tile([P, NC_, P], BF16, tag="pt")
                        nc.scalar.copy(out=pt_sb[:], in_=ptp[:])
                        pcp = ps_cp.tile([P, CE], F32, tag="cp")
                        for j in range(NC_):
                            nc.tensor.matmul(out=pcp[:], lhsT=pt_sb[:, j, :],
                                             rhs=colemb_sb[:, j, :],
                                             start=(j == 0), stop=(j == NC_ - 1))
                        nc.scalar.activation(out=y_t[:, C:D2], in_=pcp[:],
                                             func=AF.Identity, scale=recip[:])
                        st2 = stats.tile([P, nc.vector.BN_STATS_DIM], F32, tag="st2")
                        nc.vector.bn_stats(out=st2[:], in_=y_t[:])
                        nc.vector.bn_aggr(out=mv2p[:, h, :], in_=st2[:])
                    # ---- stage C: LN2 -> z2 ----
                    rstd2p = _newton_rstd(nc, stats, mv2p[:, :, 1], 2)
                    for h in range(2):
                        y_t = y_pair[h]
                        nmr = stats.tile([P, 1], F32, tag="nmr")
                        nc.vector.tensor_scalar(out=nmr[:], in0=mv2p[:, h, 0:1],
                                                scalar1=rstd2p[:, h:h + 1],
                                                scalar2=-1.0,
                                                op0=OP.mult, op1=OP.mult)
                        z2_t = z2pool.tile([P, D2], BF16, tag="z2")
                        nc.scalar.activation(out=z2_t[:], in_=y_t[:],
                                             func=AF.Identity,
                                             scale=rstd2p[:, h:h + 1], bias=nmr[:])
                        if flags["ln2w"]:
                            nc.vector.tensor_tensor(out=z2_t[:], in0=z2_t[:],
                                                    in1=bias_sb["ln2w"][:],
                                                    op=OP.mult)
                        if flags["ln2b"]:
                            nc.vector.tensor_tensor(out=z2_t[:], in0=z2_t[:],
                                                    in1=bias_sb["ln2b"][:],
                                                    op=OP.add)
                        z2_tiles.append(z2_t)

        tc.no_sync_barrier()

        # ================= pass 2: MLP (gelu) =================
        with (
            tc.tile_pool(name="p2z2T", bufs=2) as z2Tpool,
            tc.tile_pool(name="p2hT", bufs=2) as hTpool,
            tc.tile_pool(name="p2tz", bufs=2, space="PSUM") as ps_tz,
            tc.tile_pool(name="p2h", bufs=3, space="PSUM") as ps_h,
            tc.tile_pool(name="p2m", bufs=2, space="PSUM") as ps_m,
        ):
            for T in range(NT):
                z2T = z2Tpool.tile([P, DC, TW], BF16, tag="z2T")
                for f in range(DC):
                    tz = ps_tz.tile([P, TW], BF16, tag="tz")
                    for s in range(NSUB):
                        nc.tensor.transpose(
                            out=tz[:, s * P:(s + 1) * P],
                            in_=z2_tiles[T * NSUB + s][:, f * P:(f + 1) * P],
                            identity=ident_b[:])
                    nc.any.tensor_copy(out=z2T[:, f, :], in_=tz[:])
                # hT = gelu(fc1^T @ z2T): feature-major, no transpose of h
                hT = hTpool.tile([P, DC, TW], BF16, tag="hT")
                for o in range(DC):
                    ph = ps_h.tile([P, TW], F32, tag="h")
                    for f in range(DC):
                        nc.tensor.matmul(out=ph[:],
                                         lhsT=fc1_sb[:, f, o * P:(o + 1) * P],
                                         rhs=z2T[:, f, :],
                                         start=(f == 0), stop=(f == DC - 1))
                    bias = bias_sb["c1"][:, o:o + 1] if flags["c1"] else 0.0
                    nc.scalar.activation(out=hT[:, o, :], in_=ph[:], func=AF.Gelu,
                                         bias=bias)
                # mlp (token-major) + residual, v overwrites the z2 slot
                for s in range(NSUB):
                    pm = ps_m.tile([P, TW], F32, tag="m")
                    for o in range(DC):
                        nc.tensor.matmul(out=pm[:],
                                         lhsT=hT[:, o, s * P:(s + 1) * P],
                                         rhs=fc2_sb[:, o, :], start=(o == 0),
                                         stop=(o == DC - 1) and not flags["fc2b"])
                    if flags["fc2b"]:
                        nc.tensor.matmul(out=pm[:], lhsT=ones_bf[:],
                                         rhs=bias_sb["fc2b"][:],
                                         start=False, stop=True)
                    z2_t = z2_tiles[T * NSUB + s]
                    nc.vector.tensor_tensor(out=z2_t[:], in0=z2_t[:], in1=pm[:],
                                            op=OP.add)

        tc.no_sync_barrier()

        # ================= pass 3: LN3 + output conv =================
        with (
            tc.tile_pool(name="p3z3", bufs=6) as z3pool,
            tc.tile_pool(name="p3z3T", bufs=2) as z3Tpool,
            tc.tile_pool(name="p3o", bufs=4) as opool,
            tc.tile_pool(name="p3stats", bufs=4) as stats3,
            tc.tile_pool(name="p3tz", bufs=2, space="PSUM") as ps_tz3,
            tc.tile_pool(name="p3o_ps", bufs=2, space="PSUM") as ps_o,
        ):
            for T in range(NT):
                mv3 = stats3.tile([P, NSUB, 2], F32, tag="mv3")
                for s in range(NSUB):
                    st3 = stats3.tile([P, nc.vector.BN_STATS_DIM], F32, tag="st3")
                    nc.vector.bn_stats(out=st3[:], in_=z2_tiles[T * NSUB + s][:])
                    nc.vector.bn_aggr(out=mv3[:, s, :], in_=st3[:])
                rstd3 = _newton_rstd(nc, stats3, mv3[:, :, 1], NSUB)
                nmr3 = stats3.tile([P, NSUB], F32, tag="nmr3")
                nc.vector.tensor_tensor(out=nmr3[:], in0=mv3[:, :, 0],
                                        in1=rstd3[:], op=OP.mult)
                nc.vector.tensor_scalar(out=nmr3[:], in0=nmr3[:], scalar1=-1.0,
                                        scalar2=None, op0=OP.mult)
                z3_list = []
                for s in range(NSUB):
                    z3_t = z3pool.tile([P, D2], BF16, tag="z3")
                    nc.scalar.activation(out=z3_t[:],
                                         in_=z2_tiles[T * NSUB + s][:],
                                         func=AF.Identity,
                                         scale=rstd3[:, s:s + 1],
                                         bias=nmr3[:, s:s + 1])
                    z3_list.append(z3_t)
                z3T = z3Tpool.tile([P, DC, TW], BF16, tag="z3T")
                for f in range(DC):
                    tz = ps_tz3.tile([P, TW], BF16, tag="tz3")
                    for s in range(NSUB):
                        nc.tensor.transpose(
                            out=tz[:, s * P:(s + 1) * P],
                            in_=z3_list[s][:, f * P:(f + 1) * P],
                            identity=ident_b[:])
                    nc.any.tensor_copy(out=z3T[:, f, :], in_=tz[:])
                for cc in range(CC):
                    po = ps_o.tile([P, TW], F32, tag="o")
                    for f in range(DC):
                        nc.tensor.matmul(out=po[:],
                                         lhsT=conv_sb[:, f, cc * P:(cc + 1) * P],
                                         rhs=z3T[:, f, :],
                                         start=(f == 0), stop=(f == DC - 1))
                    ot = opool.tile([P, TW], F32, tag="ot")
                    if flags["ccb"]:
                        nc.any.tensor_scalar(out=ot[:], in0=po[:],
                                             scalar1=bias_sb["ccb"][:, cc:cc + 1],
                                             scalar2=None, op0=OP.add)
                    else:
                        nc.any.tensor_copy(out=ot[:], in_=po[:])
                    nc.sync.dma_start(
                        out=out_d[cc * P:(cc + 1) * P, T * TW:(T + 1) * TW],
                        in_=ot[:])

    nc.compile()
    return nc


_CACHE = {}


def _chunk(a, p=P):
    """[K, N] -> [P, K//P, N] (k-chunks on partitions)."""
    k, n = a.shape
    return np.ascontiguousarray(a.reshape(k // p, p, n).transpose(1, 0, 2))


def _prep_inputs_impl(x, cls, color_centers, semantic_centers, a_embed, b_embed,
                      ce_w, ce_b, sem_w, sem_b, q_w, q_b,
                      n1_w, n1_b, n2_w, n2_b, n3_w, n3_b,
                      fc1_w, fc1_b, fc2_w, fc2_b, conv_w, conv_b):
    f32 = lambda a: np.asarray(a, np.float32)
    bf = lambda a: np.ascontiguousarray(np.asarray(a, ml_dtypes.bfloat16))
    x = np.ascontiguousarray(f32(x))
    cls = f32(cls)
    color_centers = np.asarray(color_centers, np.int64)
    semantic_centers = f32(semantic_centers)
    a_embed, b_embed = f32(a_embed), f32(b_embed)
    ce_w, ce_b = f32(ce_w), f32(ce_b)
    sem_w, sem_b = f32(sem_w), f32(sem_b)
    q_w, q_b = f32(q_w), f32(q_b)
    n1_w, n1_b = f32(n1_w), f32(n1_b)
    n2_w, n2_b = f32(n2_w), f32(n2_b)
    n3_w, n3_b = f32(n3_w), f32(n3_b)
    fc1_w, fc1_b = f32(fc1_w), f32(fc1_b)
    fc2_w, fc2_b = f32(fc2_w), f32(fc2_b)
    conv_w, conv_b = f32(conv_w), f32(conv_b)

    # ---- host-side weight folding ----
    semP = semantic_centers @ sem_w + sem_b                  # [n, e]
    Wf = (n1_w[:, None] * q_w) @ semP.T                      # [c, n]
    ncsum = -Wf.sum(0)                                       # [n]
    crow = (n1_b @ q_w + q_b) @ semP.T                       # [n]
    ab = np.concatenate([a_embed[color_centers[:, :, 0]],
                         b_embed[color_centers[:, :, 1]]], -1)   # [4, n, 2ce]
    ce = np.einsum('inf,ifd->ind', ab, ce_w) + ce_b[:, None, :]  # [4, n, ce]
    colemb_all = np.einsum('ind,bi->bnd', ce, cls)               # [b, n, ce]
    conv_f = n3_w[:, None] * conv_w
    ccb = n3_b @ conv_w + conv_b

    nz = lambda a: bool(np.any(a != 0))
    flags = {
        "qcr": nz(crow),
        "c1": nz(fc1_b),
        "fc2b": nz(fc2_b),
        "ln2w": bool(np.any(n2_w != 1.0)),
        "ln2b": nz(n2_b),
        "ccb": nz(ccb),
    }

    wf_p = _chunk(Wf)                                        # [P, CC, NCOL] f32
    fc1_p = bf(_chunk(fc1_w))                                # [P, DC, D2]
    fc2_p = bf(_chunk(fc2_w))
    conv_p = bf(_chunk(conv_f))                              # [P, DC, C]
    ncsum_p = bf(ncsum[None, :])

    xn = x.reshape(B, C, S)
    in_maps = []
    for k in range(N_CORES):
        m = {
            "x": np.ascontiguousarray(xn[k]),
            "wf": wf_p,
            "ncsum": ncsum_p,
            "colemb": bf(_chunk(colemb_all[k])),             # [P, NC_, CE]
            "fc1": fc1_p,
            "fc2": fc2_p,
            "conv": conv_p,
        }
        if flags["qcr"]:
            m["crow"] = bf(crow[None, :])
        if flags["c1"]:
            m["c1b"] = np.ascontiguousarray(fc1_b.reshape(DC, P).T)
        if flags["fc2b"]:
            m["fc2b"] = bf(fc2_b[None, :])
        if flags["ln2w"]:
            m["ln2w"] = np.ascontiguousarray(np.broadcast_to(n2_w, (P, D2)))
        if flags["ln2b"]:
            m["ln2b"] = np.ascontiguousarray(np.broadcast_to(n2_b, (P, D2)))
        if flags["ccb"]:
            m["ccb"] = np.ascontiguousarray(ccb.reshape(CC, P).T)
        in_maps.append(m)
    return flags, in_maps


def run(flags, in_maps, **kw):
    key = tuple(sorted(flags.items()))
    if key not in _CACHE:
        _CACHE[key] = build_bass(flags)
    nc = _CACHE[key]
    res = run_bass_kernel_spmd(nc, in_maps, core_ids=list(range(N_CORES)), **kw)
    out = np.stack([res.results[k]["out"] for k in range(N_CORES)], axis=0)
    return out.reshape(B, C, H, W), res


def kernel(**inputs):
    flags, in_maps = _prep_inputs(**inputs)
    out, _ = run(flags, in_maps)
    return out


def _prep_inputs(x, cls, color_centers, semantic_centers, a_embed, b_embed,
                 ce_w, ce_b, sem_w, sem_b, q_w, q_b,
                 n1_w, n1_b, n2_w, n2_b, n3_w, n3_b,
                 fc1_w, fc1_b, fc2_w, fc2_b, conv_w, conv_b):
    return _prep_inputs_impl(
        x, cls, color_centers, semantic_centers, a_embed, b_embed,
        ce_w, ce_b, sem_w, sem_b, q_w, q_b,
        n1_w, n1_b, n2_w, n2_b, n3_w, n3_b,
        fc1_w, fc1_b, fc2_w, fc2_b, conv_w, conv_b)
